# revision 32
# baseline (speedup 1.0000x reference)
"""Trainium2 Bass kernel for nn_IterativeClassifier (B=65536, D=512, E=64, C=10, T=40).

Strategy (pure data parallel over 8 cores, batch-sharded):
  All activations live TRANSPOSED on-chip: [E, batch] with batch on the free dim.
  The z-state is never materialized. Using relu positive-homogeneity and the
  de-scaled substitution  h^_t := 0.9^-t * h_t, the whole 40-step recurrence
  becomes a pair of persistent PSUM accumulators per batch tile:
    HA_t = 0.9^-t * (W1f@F + W1z@z_t)    (PSUM, matmul-accumulated)
    L    = logits accumulator            (PSUM, matmul-accumulated)
  Per step and batch-tile only THREE small matmuls (K=64,M<=64, quadrant-tiled
  across the PE array) and ONE PSUM->SBUF evacuation (relu+bias, alternating
  ScalarE/VectorE) are needed:
    HA += (0.1/0.9 * W1z@W2) @ h^_{t-1}      (mm_a)
    L  += (0.1 * CE@W2) @ h^_{t-1}           (mm_L)
    HA += (0.1 * 0.9^-t * W1f) @ F           (mm_b, per-step prescaled weights)
    h^_t = relu(HA + beta_t)                 (evac)
  Finally logits = 0.9^39 * L + biasL.

  All matmul operands are bf16 (4x PE throughput vs fp32; fp32 PSUM
  accumulation; rel-err ~2e-3 vs 2e-2 budget). mm_L quadrants alternate by
  pair parity so all four 64x64 PE quadrant streams carry an equal load
  (6 matmuls/quadrant/step/chunk instead of 4/4/8/8).

  Per core: 16 batch tiles of 512 columns, paired onto 128 partitions
  (tile A on partitions 0:64, tile B on 64:128), 2 chunks of 4 pairs
  (PSUM: 4 HA banks + 4 L banks = all 8 banks per chunk).
"""

import ml_dtypes
import numpy as np

import concourse.bass as bass
import concourse.bacc as bacc
import concourse.mybir as mybir
import concourse.tile as tile
from concourse.bass_utils import run_bass_kernel_spmd

F32 = mybir.dt.float32
BF16 = mybir.dt.bfloat16
AF = mybir.ActivationFunctionType
ALU = mybir.AluOpType

NCORES = 8
B, D, E, C, T = 65536, 512, 64, 10, 40
DEC, LR = 0.9, 0.1
NT = 512                      # batch columns per tile
BSH = B // NCORES             # 8192 batch rows per core
TILES = BSH // NT             # 16
PAIRS = TILES // 2            # 8
CHUNK_PAIRS = 4
CHUNKS = PAIRS // CHUNK_PAIRS # 2

NBF = ml_dtypes.bfloat16


def _dedup_ldweights(nc):
    """Drop InstLdweights that reload the exact weights already resident in
    the same PE array region (tile_position+tile_size), with no intervening
    load touching an overlapping region. Legalization emits one LDWEIGHTS per
    matmul unconditionally; consecutive same-weight matmuls on a quadrant
    (e.g. G' applied to every pair in a chunk) don't need the reload, and on
    hardware an LDWEIGHTS cannot overlap matmuls sharing its row group, so
    each elided load saves ~100ns of PE issue time."""
    def fp_of(ap):
        return (ap.memref, ap.offset, str(ap.ap), str(ap.dtype))

    removed = 0
    for blk in nc.main_func.blocks:
        loaded = {}  # (tile_position, tile_size) -> weights fingerprint
        keep = []
        for inst in blk.instructions:
            if isinstance(inst, mybir.InstLdweights):
                si = inst.sync_info
                clean = (si is None or (not si.on_wait and not si.on_update)) \
                    and not inst.descendants and not inst.is_transpose \
                    and inst.perf_mode is None
                tp = inst.tile_position
                ts = inst.tile_size
                key = (tuple(tp) if tp else None, tuple(ts) if ts else None)
                fp = fp_of(inst.ins[0])
                if clean and key[0] is not None and key[1] is not None \
                        and loaded.get(key) == fp:
                    removed += 1
                    continue
                if key[0] is None or key[1] is None:
                    loaded.clear()
                else:
                    r0, c0 = key[0]; rs, cs = key[1]
                    for (otp, ots) in list(loaded):
                        if otp is None:
                            del loaded[(otp, ots)]; continue
                        orow, ocol = otp; ors, ocs = ots
                        if r0 < orow + ors and orow < r0 + rs \
                                and c0 < ocol + ocs and ocol < c0 + cs:
                            del loaded[(otp, ots)]
                    loaded[key] = fp
                keep.append(inst)
            else:
                if isinstance(inst, mybir.InstMatmult):
                    ifmap = inst.ins[0]
                    if str(ifmap.dtype) in ("dt.float32", "dt.float32r") \
                            or inst.is_transpose:
                        loaded.clear()  # self-loading matmul clobbers weights
                keep.append(inst)
        blk.instructions[:] = keep
    return removed


def _host_prep(x, z0, W_feat, b_feat, W1, b1, W2, b2, class_emb):
    """All host-side numpy preprocessing: transposed/prescaled weights + shards."""
    f4 = np.float32
    W1f = W1[:, :E].astype(f4)
    W1z = W1[:, E:2 * E].astype(f4)
    w1t = W1[:, 2 * E].astype(f4)

    def dup(a):  # stack a [64,m] lhsT onto both partition halves -> [128,m]
        return np.concatenate([a, a], axis=0).astype(f4)

    Gp = (LR / DEC) * (W1z @ W2)                       # [64,64]
    CL = LR * (class_emb @ W2)                         # [10,64]
    CEi = DEC * class_emb                              # [10,64]
    wf_blocks = [W1f.T] + [(LR * DEC ** (-t)) * W1f.T for t in range(1, T)]
    wf = dup(np.concatenate(wf_blocks, axis=1))        # [128, 40*64]
    wg = dup(Gp.T)                                     # [128, 64]
    w1zbd = np.zeros((128, 128), f4)                   # blockdiag: one full-bank init
    w1zbd[0:E, 0:E] = W1z.T
    w1zbd[E:128, E:128] = W1z.T
    clp = np.zeros((E, E), f4); clp[:, :C] = CL.T
    cl = dup(clp)                                      # [128, 64]
    ceanti = np.zeros((128, 128), f4)                  # anti-blockdiag L init
    ceanti[0:E, E:E + C] = CEi.T
    ceanti[E:128, 0:C] = CEi.T
    wfeat = W_feat.T.reshape(4, 128, E).transpose(1, 0, 2).reshape(128, 4 * E).astype(f4)
    # wfeat[p, 64k+m] = W_feat.T[128k+p, m] -> slice [:, 64k:64k+64] is chunk k
    beta = np.stack([
        DEC ** (-t) * (b1 + (t / T) * w1t + (1 - DEC ** t) * (W1z @ b2) + W1f @ b_feat)
        for t in range(T)
    ]).T.astype(f4)                                    # [64, 40]
    beta = np.concatenate([beta, beta], axis=0)        # [128, 40]
    biasl = np.zeros((128, 1), f4)
    bl = ((1 - DEC ** T) * (class_emb @ b2)).astype(f4)
    biasl[0:C, 0] = bl
    biasl[64:64 + C, 0] = bl

    # x -> per-core per-tile [128, 4*NT] blocks:  x_dev[c,i,p,k*NT+n] = x[c*BSH+i*NT+n, 128k+p]
    xr = x.astype(f4).reshape(NCORES, TILES, NT, 4, 128).transpose(0, 1, 4, 3, 2)
    x_dev = np.ascontiguousarray(xr.reshape(NCORES, TILES, 128, 4 * NT)).astype(NBF)
    # z0 -> per-core per-pair [128, NT]: rows 0:64 = tile 2p, rows 64:128 = tile 2p+1
    zr = z0.astype(f4).reshape(NCORES, PAIRS, 2, NT, E).transpose(0, 1, 2, 4, 3)
    z0_dev = np.ascontiguousarray(zr.reshape(NCORES, PAIRS, 128, NT)).astype(NBF)

    consts = np.concatenate(
        [wfeat, wg, wf, w1zbd, cl, ceanti], axis=1).astype(NBF)
    consts32 = np.concatenate([beta, biasl], axis=1).astype(f4)
    return {"consts_d": consts, "consts32_d": consts32}, x_dev, z0_dev


def build(n_tiles=TILES, t_steps=T):
    """Build the Bass module. Returns nc."""
    n_pairs = n_tiles // 2
    chunk_pairs = min(CHUNK_PAIRS, n_pairs)
    nc = bacc.Bacc("TRN2", target_bir_lowering=False, debug=False)

    x_d = nc.dram_tensor("x_d", [n_tiles, 128, 4 * NT], BF16, kind="ExternalInput").ap()
    z0_d = nc.dram_tensor("z0_d", [n_pairs, 128, NT], BF16, kind="ExternalInput").ap()
    NCONST = 4 * E + E + T * E + 128 + E + 128
    consts_d = nc.dram_tensor("consts_d", [128, NCONST], BF16, kind="ExternalInput").ap()
    consts32_d = nc.dram_tensor("consts32_d", [128, T + 1], F32, kind="ExternalInput").ap()
    out_d = nc.dram_tensor("out_d", [n_tiles, C, NT], F32, kind="ExternalOutput").ap()

    scale_l = float(DEC ** (t_steps - 1))

    with tile.TileContext(nc) as tc:
        with (
            tc.sbuf_pool(name="consts", bufs=1) as cpool,
            tc.sbuf_pool(name="consts32", bufs=1) as cpool32,
            tc.sbuf_pool(name="xt", bufs=8) as xpool,
            tc.sbuf_pool(name="ff", bufs=chunk_pairs) as ffpool,
            tc.sbuf_pool(name="hh", bufs=2 * chunk_pairs) as hhpool,
            tc.sbuf_pool(name="z0s", bufs=2) as zpool,
            tc.sbuf_pool(name="ll", bufs=2) as llpool,
            tc.psum_pool(name="ha", bufs=chunk_pairs) as hapool,
            tc.psum_pool(name="lb", bufs=chunk_pairs) as lbpool,
        ):
            const_sb = cpool.tile([128, NCONST], BF16)
            # wfeat first as its own small transfer so the feature matmuls
            # aren't gated on the whole 0.8MB const block
            nc.sync.dma_start(const_sb[:, 0:4 * E], consts_d[:, 0:4 * E])
            nc.sync.dma_start(const_sb[:, 4 * E:], consts_d[:, 4 * E:])
            const32_sb = cpool32.tile([128, T + 1], F32)
            nc.sync.dma_start(const32_sb, consts32_d)
            o = 0
            def _sl(n):
                nonlocal o
                v = const_sb[:, o:o + n]; o += n; return v
            wfeat_sb = _sl(4 * E); wg_sb = _sl(E); wf_sb = _sl(T * E)
            w1zbd_sb = _sl(128); cl_sb = _sl(E); ceanti_sb = _sl(128)
            beta_sb = const32_sb[:, 0:T]
            biasl_sb = const32_sb[:, T:T + 1]

            LO, HI = slice(0, 64), slice(64, 128)

            HNT = NT // 2

            def evac_split(dst, src, bias_ap):
                # column-split across ScalarE+VectorE: ~half the latency.
                # Used for the pair whose evac gates the next step's matmuls.
                nc.scalar.activation(dst[:, 0:HNT], src[:, 0:HNT], AF.Relu,
                                     bias=bias_ap, scale=1.0)
                nc.vector.tensor_scalar(dst[:, HNT:NT], src[:, HNT:NT], bias_ap,
                                        0.0, ALU.add, ALU.max)

            def evac_full(on_scalar, dst, src, bias_ap):
                if on_scalar:
                    nc.scalar.activation(dst, src, AF.Relu, bias=bias_ap, scale=1.0)
                else:
                    nc.vector.tensor_scalar(dst, src, bias_ap, 0.0, ALU.add, ALU.max)

            def evac_split2(dst, src, bias_ap):
                # mirror split (DVE low half, Scalar high) to balance engines
                nc.vector.tensor_scalar(dst[:, 0:HNT], src[:, 0:HNT], bias_ap,
                                        0.0, ALU.add, ALU.max)
                nc.scalar.activation(dst[:, HNT:NT], src[:, HNT:NT], AF.Relu,
                                     bias=bias_ap, scale=1.0)

            def evac(pi, dst, src, bias_ap, t=0):
                # full single-engine evacs, alternating Scalar/Vector: fewer
                # per-op overheads and semaphore hops than splitting.
                evac_full(pi % 2 == 0, dst, src, bias_ap)

            def mm_L(p, lb, hh, stop=False):
                # anti-diagonal quadrants (the diagonal ones carry mm_a+mm_b):
                # L_A at LB rows 64:64+C, L_B at rows 0:C.
                nc.tensor.matmul(lb[HI], cl_sb[LO], hh[LO],
                                 start=False, stop=stop, skip_group_check=True)
                nc.tensor.matmul(lb[LO], cl_sb[HI], hh[HI],
                                 start=False, stop=stop, skip_group_check=True)

            for chunk in range((n_pairs + chunk_pairs - 1) // chunk_pairs):
                pairs = range(chunk * chunk_pairs,
                              min((chunk + 1) * chunk_pairs, n_pairs))
                HA, LB, FF, HH = {}, {}, {}, {}
                # ---- feature + init phase ----
                for p in pairs:
                    fp = hapool.tile([128, NT], F32, tag="ha", name=f"fp{p}")
                    for ab in range(2):  # ab=0 -> tile A=2p (F at LO), ab=1 -> B (F at HI)
                        dst = fp[LO] if ab == 0 else fp[HI]
                        for k in range(4):
                            xt = xpool.tile([128, NT], BF16, tag="xt",
                                            name=f"xt{p}_{ab}_{k}")
                            # 4-way trigger spread; scalar/vector are idle
                            # during the feature phase, and this keeps the
                            # x stream ahead of the feature matmuls
                            dma_eng = (nc.gpsimd, nc.sync, nc.scalar,
                                       nc.gpsimd)[(ab * 4 + k) % 4]
                            dma_eng.dma_start(xt, x_d[2 * p + ab, :, NT * k:NT * (k + 1)])
                            nc.tensor.matmul(dst, wfeat_sb[:, E * k:E * (k + 1)],
                                             xt, start=(k == 0), stop=(k == 3), skip_group_check=True)
                    ff = ffpool.tile([128, NT], BF16, tag="ff", name=f"ff{p}")
                    nc.scalar.activation(ff, fp, AF.Copy, bias=0.0, scale=1.0)
                    FF[p] = ff

                    z0t = zpool.tile([128, NT], BF16, tag="z0s", name=f"z0t{p}")
                    nc.gpsimd.dma_start(z0t, z0_d[p])
                    ha = hapool.tile([128, NT], F32, tag="ha", name=f"ha{p}")
                    lb = lbpool.tile([128, NT], F32, tag="lb", name=f"lb{p}")
                    HA[p], LB[p] = ha, lb
                    # HA_0 = W1z@z0, quadrant-tiled (the blockdiag splits into
                    # two 64x64 diagonal-quadrant matmuls — full-array matmuls
                    # would block every PE region and stall the init pipeline)
                    nc.tensor.matmul(ha[LO], w1zbd_sb[0:64, 0:64], z0t[LO],
                                     start=True, stop=False, skip_group_check=True)
                    nc.tensor.matmul(ha[HI], w1zbd_sb[64:128, 64:128], z0t[HI],
                                     start=True, stop=False, skip_group_check=True)
                    nc.tensor.matmul(ha[LO], wf_sb[LO, 0:E], ff[LO], start=False, stop=False, skip_group_check=True)
                    nc.tensor.matmul(ha[HI], wf_sb[HI, 0:E], ff[HI], start=False, stop=True, skip_group_check=True)
                    # L_init = (0.9*CE)@z0, anti-quadrant-tiled (A -> L[HI], B -> L[LO])
                    nc.tensor.matmul(lb[HI], ceanti_sb[0:64, 64:128], z0t[LO],
                                     start=True, stop=False, skip_group_check=True)
                    nc.tensor.matmul(lb[LO], ceanti_sb[64:128, 0:64], z0t[HI],
                                     start=True, stop=False, skip_group_check=True)
                    hh = hhpool.tile([128, NT], BF16, tag="hh", name=f"hh{p}_0")
                    evac(p % chunk_pairs, hh, ha, beta_sb[:, 0:1])
                    HH[p] = hh

                # ---- 39 recurrence steps ----
                # mm_a and mm_b write the same PSUM elements; both run on the
                # SAME PE quadrant per lane ((0,0) for the LO lane, (64,64)
                # for HI) so the hardware serializes them — a cross-quadrant
                # schedule races the accumulating writes and crashes the
                # device. mm_L streams on the otherwise-idle anti-diagonal
                # quadrants; evacs (split Scalar+Vector) close each pair.
                for t in range(1, t_steps):
                    def mm_a(p, stop=False):
                        nc.tensor.matmul(HA[p][LO], wg_sb[LO], HH[p][LO],
                                         start=False, stop=False, skip_group_check=True)
                        nc.tensor.matmul(HA[p][HI], wg_sb[HI], HH[p][HI],
                                         start=False, stop=stop, skip_group_check=True)
                    def mm_b(p, stop=True):
                        nc.tensor.matmul(HA[p][LO], wf_sb[LO, E * t:E * (t + 1)],
                                         FF[p][LO], start=False, stop=False, skip_group_check=True)
                        nc.tensor.matmul(HA[p][HI], wf_sb[HI, E * t:E * (t + 1)],
                                         FF[p][HI], start=False, stop=stop, skip_group_check=True)
                    # First pair's a+b issue back-to-back so its evac (which
                    # gates the next step's first matmul) starts ~0.7us into
                    # the step; the other pairs stay grouped so LDWEIGHTS
                    # dedup keeps working (4 loads/step not 8).
                    # [a1,b1, b2..b4, a2..a4]: only TWO weight reloads per
                    # step survive dedup (a1->b1, b4->a2); the step boundary
                    # a4 -> a1' reuses G' with no reload.
                    plist = list(pairs)
                    mm_a(plist[0]); mm_b(plist[0])
                    for p in plist[1:]:
                        mm_b(p, stop=False)
                    for p in plist[1:]:
                        mm_a(p, stop=True)
                    for p in pairs:  # mm_L on anti-diagonal (reads h^_{t-1})
                        mm_L(p, LB[p], HH[p])
                    for p in pairs:
                        hh = hhpool.tile([128, NT], BF16, tag="hh", name=f"hh{p}_{t}")
                        evac(p % chunk_pairs, hh, HA[p], beta_sb[:, t:t + 1], t)
                        HH[p] = hh

                # ---- final: last mm_L, logits evac + store ----
                for p in pairs:
                    mm_L(p, LB[p], HH[p], stop=True)
                    ll = llpool.tile([128, NT], F32, tag="ll", name=f"ll{p}")
                    nc.scalar.activation(ll, LB[p], AF.Identity,
                                         bias=biasl_sb[:, 0:1], scale=scale_l)
                    nc.sync.dma_start(out_d[2 * p], ll[64:64 + C, :])
                    nc.gpsimd.dma_start(out_d[2 * p + 1], ll[0:C, :])
    # Dedup must run AFTER move_matmul_waits_to_ldweights (which runs inside
    # compile): that pass moves excess matmul waits to the most recent
    # LDWEIGHTS, and if dedup already removed the adjacent one the wait lands
    # on a much earlier load — deadlock. Hook in right after it instead.
    orig_mv = nc.move_matmul_waits_to_ldweights
    def _mv_then_dedup():
        orig_mv()
        _dedup_ldweights(nc)
    nc.move_matmul_waits_to_ldweights = _mv_then_dedup
    nc.compile()
    return nc


_BUILT = {}


def _get_nc(n_tiles=TILES, t_steps=T):
    key = (n_tiles, t_steps)
    if key not in _BUILT:
        _BUILT[key] = build(n_tiles, t_steps)
    return _BUILT[key]


def kernel(x, z0, W_feat, b_feat, W1, b1, W2, b2, class_emb, T_steps, **run_kw):
    x = np.asarray(x); z0 = np.asarray(z0)
    assert int(T_steps) == T
    const, x_dev, z0_dev = _host_prep(
        np.asarray(x), np.asarray(z0), np.asarray(W_feat), np.asarray(b_feat),
        np.asarray(W1), np.asarray(b1), np.asarray(W2), np.asarray(b2),
        np.asarray(class_emb))
    nc = _get_nc()
    in_maps = []
    for c in range(NCORES):
        m = dict(const)
        m["x_d"] = x_dev[c]
        m["z0_d"] = z0_dev[c]
        in_maps.append(m)
    res = run_bass_kernel_spmd(nc, in_maps, core_ids=list(range(NCORES)), **run_kw)
    outs = [r["out_d"] for r in res.results]  # each [TILES, C, NT]
    # out[c][i, cc, n] -> logits[c*BSH + i*NT + n, cc]
    stacked = np.stack(outs)                       # [8, 16, 10, 512]
    logits = stacked.transpose(0, 1, 3, 2).reshape(B, C)
    if run_kw:
        kernel.last_result = res
    return np.ascontiguousarray(logits.astype(np.float32))
